# Initial kernel scaffold
#
"""MixedArityTreeLSTM Trainium2 kernel.

Level-synchronous bottom-up Tree-LSTM over B=256 heap-indexed perfect binary
trees (511 nodes, depth 8), E=H=128. Pure data-parallel over 8 NeuronCores
(32 trees per core); all weights replicated.

Per-core layout: activations stored feature-major [H(part), nodes(free)].
Heap order makes left/right children the even/odd columns of the child level.
Binary/unary arity blending is folded into the matmuls via masked children:
    pre_g = W_g^T x + Ubt_g^T (m*h_l) + Ubb_g^T (m*h_r) + Uun_g^T ((1-m)*h_l)
            + m * (b_bin_g - b_un_g)   [K=1 outer-product matmul]
            + (bW_g + b_un_g)          [ACT bias]
Matmul operands are bf16 (2 col/cycle on PE); PSUM/h/c/gates stay fp32.
The embedding gather uses dma_gather(transpose=True) on a bf16 embedding
table, which lands x^T (feature-major) in SBUF directly.
"""

import os

import numpy as np
import ml_dtypes

# debug knobs (bisection); full kernel when unset
DBG_MIN_LVL = int(os.environ.get("TL_MIN_LVL", "0"))  # stop after this level
DBG_NO_DELTA = os.environ.get("TL_NO_DELTA", "") == "1"
N_QUEUES = int(os.environ.get("TL_NQ", "2"))

B, D = 256, 8
V, E, H = 32000, 128, 128
N_NODES = 2 ** (D + 1) - 1  # 511
NCORES = 8
BL = B // NCORES  # 32 trees per core

# levels in processing order: leaves (l=8) then 7..0
# (lvl, n real cols, Pw padded cols)
LEVELS = [(l, BL * (2**l), max(128, BL * (2**l))) for l in range(D, -1, -1)]
LVL_N = {l: BL * (2**l) for l in range(D + 1)}
LVL_PW = {l: max(128, BL * (2**l)) for l in range(D + 1)}

# chunks per level (chunk = up to 512 cols)
CPL = {l: max(1, LVL_N[l] // 512) for l in range(D + 1)}

# post-order dependency wave over the chunk tree: children before parent
ORDER = []


def _post(l, j):
    if l < D:
        if CPL[l + 1] == 2 * CPL[l]:
            _post(l + 1, 2 * j)
            _post(l + 1, 2 * j + 1)
        else:
            assert CPL[l + 1] == CPL[l] == 1
            _post(l + 1, 0)
    ORDER.append((l, j))


_post(0, 0)

# gather calls in wave order: (lvl, col0 within level's padded xT, width)
GATHER_CALLS = [
    (lvl, j * 512, min(512, LVL_PW[lvl] - j * 512)) for lvl, j in ORDER
]

# internal-level compute chunks in wave order: (cid, lvl, c0, N, mask offset)
CHUNKS = []
_moff = 0
for lvl, j in ORDER:
    if lvl == D:
        continue
    N = min(512, LVL_N[lvl] - j * 512)
    CHUNKS.append((len(CHUNKS), lvl, j * 512, N, _moff))
    _moff += N
N_MASK_ROWS = len(CHUNKS)  # 19
MASKB_LEN = _moff  # 8160

IDX_COLS = sum(w // 16 for _, _, w in GATHER_CALLS)  # 1032

BF16 = ml_dtypes.bfloat16

_CACHE = {}


def _build_nc():
    """Build the (SPMD, per-core) Bass/Tile kernel. Cached per process."""
    if "nc" in _CACHE:
        return _CACHE["nc"]

    from contextlib import ExitStack

    import concourse.mybir as mybir
    import concourse.tile as tile
    from concourse import bacc

    dt = mybir.dt
    AF = mybir.ActivationFunctionType

    nc = bacc.Bacc(num_swdge_queues=N_QUEUES)

    emb_d = nc.dram_tensor("emb_bf", [V, E], dt.bfloat16, kind="ExternalInput")
    idx_d = nc.dram_tensor("gidx", [128, IDX_COLS], dt.int16, kind="ExternalInput")
    mbc_d = nc.dram_tensor(
        "mbcast", [128, MASKB_LEN], dt.bfloat16, kind="ExternalInput"
    )
    maskb_d = nc.dram_tensor(
        "maskb", [1, MASKB_LEN], dt.bfloat16, kind="ExternalInput"
    )
    w_d = nc.dram_tensor("w_bf", [4, E, H], dt.bfloat16, kind="ExternalInput")
    ubt_d = nc.dram_tensor("ubt_bf", [5, H, H], dt.bfloat16, kind="ExternalInput")
    ubb_d = nc.dram_tensor("ubb_bf", [5, H, H], dt.bfloat16, kind="ExternalInput")
    uun_d = nc.dram_tensor("uun_bf", [4, H, H], dt.bfloat16, kind="ExternalInput")
    # bias rows: 0=b_leaf 1=bc_i 2=bc_fL 3=b_fR 4=bc_o 5=bc_u
    bias_d = nc.dram_tensor("biases", [6, H], dt.float32, kind="ExternalInput")
    # delta rows: 0=d_i 1=d_fL 2=d_o 3=d_u 4=+40 (f_r unary kill)
    delt_d = nc.dram_tensor("deltas", [5, H], dt.bfloat16, kind="ExternalInput")

    h_out_d = nc.dram_tensor("h_out", [H, BL], dt.float32, kind="ExternalOutput")
    c_out_d = nc.dram_tensor("c_out", [H, BL], dt.float32, kind="ExternalOutput")

    with tile.TileContext(nc) as tc, ExitStack() as ctx:
        consts = ctx.enter_context(tc.tile_pool(name="consts", bufs=1))

        w_sb = consts.tile([E, 4, H], dt.bfloat16)
        nc.sync.dma_start(out=w_sb, in_=w_d[:, :, :].rearrange("g e h -> e g h"))
        ubt_sb = consts.tile([H, 5, H], dt.bfloat16)
        nc.sync.dma_start(out=ubt_sb, in_=ubt_d[:, :, :].rearrange("g k h -> k g h"))
        ubb_sb = consts.tile([H, 5, H], dt.bfloat16)
        nc.sync.dma_start(out=ubb_sb, in_=ubb_d[:, :, :].rearrange("g k h -> k g h"))
        uun_sb = consts.tile([H, 4, H], dt.bfloat16)
        nc.sync.dma_start(out=uun_sb, in_=uun_d[:, :, :].rearrange("g k h -> k g h"))
        bias_sb = consts.tile([H, 6], dt.float32)
        nc.sync.dma_start(out=bias_sb, in_=bias_d[:, :].rearrange("n h -> h n"))
        delt_sb = consts.tile([1, 5, H], dt.bfloat16)
        nc.sync.dma_start(out=delt_sb, in_=delt_d[:, :].rearrange("(o g) h -> o g h", o=1))
        idx_sb = consts.tile([128, IDX_COLS], dt.int16)
        nc.sync.dma_start(out=idx_sb, in_=idx_d[:, :])
        mbc_sb = consts.tile([128, MASKB_LEN], dt.bfloat16)
        nc.sync.dma_start(out=mbc_sb, in_=mbc_d[:, :])
        maskb_sb = consts.tile([1, MASKB_LEN], dt.bfloat16)
        nc.sync.dma_start(out=maskb_sb, in_=maskb_d[:, :])

        # --- per-level xT tiles + all gathers issued up front ---
        lev = ctx.enter_context(tc.tile_pool(name="lev", bufs=1))
        xt = {}
        for lvl, n, pw in LEVELS:
            xt[lvl] = lev.tile(
                [128, pw], dt.bfloat16, name=f"xTl{lvl}", tag=f"xTl{lvl}"
            )


        # --- working pools ---
        psum = ctx.enter_context(tc.tile_pool(name="psum", bufs=8, space="PSUM"))
        work = ctx.enter_context(tc.tile_pool(name="work", bufs=2))

        h_t = {}
        c_t = {}
        n8 = LVL_N[D]
        h_t[D] = lev.tile([H, n8], dt.bfloat16, name="h_leaf", tag="h_leaf")

        # wave loop: for each chunk in post-order, gather its x then compute
        icols = {}
        _ic = 0
        for gi_, (lvl, c0, width) in enumerate(GATHER_CALLS):
            icols[(lvl, c0)] = (_ic, width, gi_)
            _ic += width // 16

        cid_of = {(lvl, c0): (cid, N, moff) for cid, lvl, c0, N, moff in CHUNKS}

        for lvl, j in ORDER:
            g0 = j * 512
            _icol, width, gi_ = icols[(lvl, g0)]
            out_view = xt[lvl][:, g0 : g0 + width].rearrange(
                "p (o n) -> p o n", o=1
            )
            nc.gpsimd.dma_gather(
                out_view,
                emb_d[:, :],
                idx_sb[:, _icol : _icol + width // 16],
                width,
                width,
                E,
                transpose=True,
                queue_num=gi_ % N_QUEUES,
            )

            if lvl == D:
                # leaf chunk: h = tanh(W3^T x + b3)
                ps = psum.tile([H, width], dt.float32, tag="pg", name="ps_leaf")
                nc.tensor.matmul(
                    ps, w_sb[:, 3, :], xt[D][:, g0 : g0 + width],
                    start=True, stop=True,
                )
                nc.scalar.activation(
                    h_t[D][:, g0 : g0 + width], ps, AF.Tanh, bias=bias_sb[:, 0:1]
                )
                continue

            cid, N, moff = cid_of[(lvl, g0)]
            c0 = g0
            if lvl < DBG_MIN_LVL:
                continue
            first_chunk = c0 == 0
            if first_chunk:
                n = LVL_N[lvl]
                hdt = dt.float32 if lvl == 0 else dt.bfloat16
                h_t[lvl] = lev.tile([H, n], hdt, name=f"h_l{lvl}", tag=f"h_l{lvl}")
                c_t[lvl] = lev.tile(
                    [H, n], dt.float32, name=f"c_l{lvl}", tag=f"c_l{lvl}"
                )

            hch = h_t[lvl + 1]
            pairs = hch[:, 2 * c0 : 2 * c0 + 2 * N].rearrange(
                "p (n two) -> p n two", two=2
            )
            h_e, h_o = pairs[:, :, 0], pairs[:, :, 1]

            mb = mbc_sb[:, moff : moff + N]

            heb = work.tile([128, N], dt.bfloat16, tag="heb", name="heb")
            nc.vector.tensor_mul(heb, h_e, mb)
            hob = work.tile([128, N], dt.bfloat16, tag="hob", name="hob")
            nc.vector.tensor_mul(hob, h_o, mb)
            heu = work.tile([128, N], dt.bfloat16, tag="heu", name="heu")
            nc.vector.tensor_sub(heu, h_e, heb)

            xs = xt[lvl][:, c0 : c0 + N]
            mrow = maskb_sb[:, moff : moff + N]
            top = lvl == D - 1  # children are leaves: c=0, skip f gates

            # gate -> (W idx, Ubin idx, Uun idx or None, delta idx or None)
            if top:
                gates = [("i", 0, 0, 0, 0), ("o", 2, 3, 2, 2), ("u", 3, 4, 3, 3)]
            else:
                gates = [
                    ("i", 0, 0, 0, 0),
                    ("fl", 1, 1, 1, 1),
                    ("fr", 1, 2, None, 4),
                    ("o", 2, 3, 2, 2),
                    ("u", 3, 4, 3, 3),
                ]

            pts = {}
            for gname, wi, ubi, uui, di in gates:
                ps = psum.tile([H, N], dt.float32, tag="pg", name=f"ps_{gname}")
                nc.tensor.matmul(ps, w_sb[:, wi, :], xs, start=True, stop=False)
                nc.tensor.matmul(ps, ubt_sb[:, ubi, :], heb, start=False, stop=False)
                nc.tensor.matmul(
                    ps, ubb_sb[:, ubi, :], hob,
                    start=False, stop=(DBG_NO_DELTA and uui is None),
                )
                if uui is not None:
                    nc.tensor.matmul(
                        ps, uun_sb[:, uui, :], heu, start=False, stop=DBG_NO_DELTA
                    )
                if not DBG_NO_DELTA:
                    nc.tensor.matmul(
                        ps, delt_sb[:, di, :], mrow, start=False, stop=True
                    )
                pts[gname] = ps

            gi = work.tile([128, N], dt.float32, tag="gi", name="gi")
            nc.scalar.activation(gi, pts["i"], AF.Sigmoid, bias=bias_sb[:, 1:2])
            go = work.tile([128, N], dt.float32, tag="go", name="go")
            nc.scalar.activation(go, pts["o"], AF.Sigmoid, bias=bias_sb[:, 4:5])
            gu = work.tile([128, N], dt.float32, tag="gu", name="gu")
            nc.scalar.activation(gu, pts["u"], AF.Tanh, bias=bias_sb[:, 5:6])

            cs = c_t[lvl][:, c0 : c0 + N]
            if top:
                nc.vector.tensor_mul(cs, gi, gu)
            else:
                gfl = work.tile([128, N], dt.float32, tag="gfl", name="gfl")
                nc.scalar.activation(
                    gfl, pts["fl"], AF.Sigmoid, bias=bias_sb[:, 2:3]
                )
                gfr = work.tile([128, N], dt.float32, tag="gfr", name="gfr")
                nc.scalar.activation(
                    gfr, pts["fr"], AF.Sigmoid, bias=bias_sb[:, 3:4]
                )
                cch = c_t[lvl + 1]
                cpairs = cch[:, 2 * c0 : 2 * c0 + 2 * N].rearrange(
                    "p (n two) -> p n two", two=2
                )
                c_e, c_o = cpairs[:, :, 0], cpairs[:, :, 1]

                t1 = work.tile([128, N], dt.float32, tag="t1", name="t1")
                nc.vector.tensor_mul(t1, gi, gu)
                t2 = work.tile([128, N], dt.float32, tag="t2", name="t2")
                nc.vector.tensor_mul(t2, gfl, c_e)
                nc.vector.tensor_add(cs, t1, t2)
                t3 = work.tile([128, N], dt.float32, tag="t3", name="t3")
                nc.vector.tensor_mul(t3, gfr, c_o)
                nc.vector.tensor_add(cs, cs, t3)

            tch = work.tile([128, N], dt.float32, tag="tch", name="tch")
            nc.scalar.activation(tch, cs, AF.Tanh)
            nc.vector.tensor_mul(h_t[lvl][:, c0 : c0 + N], go, tch)

        ol = DBG_MIN_LVL
        h_fin = h_t[ol][:, :BL]
        c_fin = c_t[ol][:, :BL] if ol in c_t else h_t[ol][:, :BL]
        eng = nc.sync if ol == 0 else nc.gpsimd
        eng.dma_start(out=h_out_d[:, :], in_=h_fin)
        eng.dma_start(out=c_out_d[:, :], in_=c_fin)

    nc.finalize()
    _CACHE["nc"] = nc
    return nc


def _wrap_idx(seg):
    """dma_gather index layout: unwrapped[i] = idxs[i % 16, i // 16],
    replicated across the 128 partitions."""
    w = seg.reshape(-1, 16).T.astype(np.int16)  # [16, len/16]
    return np.tile(w, (8, 1))


def prep_core_inputs(tokens_c, arity_c, shared):
    """Per-core input map. tokens_c [BL,511], arity_c [BL,255]."""
    idx_cols = []
    for lvl, c0, width in GATHER_CALLS:
        off, cnt = 2**lvl - 1, 2**lvl
        flat = np.asarray(tokens_c[:, off : off + cnt]).reshape(-1)
        pw = LVL_PW[lvl]
        if pw != flat.size:
            flat = np.concatenate([flat, np.zeros(pw - flat.size, np.int64)])
        idx_cols.append(_wrap_idx(flat[c0 : c0 + width]))
    gidx = np.concatenate(idx_cols, axis=1)
    assert gidx.shape == (128, IDX_COLS)

    maskb = np.zeros((1, MASKB_LEN), BF16)
    for cid, lvl, c0, N, moff in CHUNKS:
        off = 2**lvl - 1
        m = (
            (np.asarray(arity_c[:, off : off + 2**lvl]).reshape(-1) == 1)
            .astype(np.float32)
        )
        maskb[0, moff : moff + N] = m[c0 : c0 + N].astype(BF16)

    return dict(
        shared,
        gidx=gidx,
        maskb=maskb,
        mbcast=np.broadcast_to(maskb, (128, MASKB_LEN)).copy(),
    )


def prep_shared_inputs(emb, W, bW, Ubin, bUbin, Uun, bUun):
    emb = np.asarray(emb, np.float32)
    W = np.asarray(W, np.float32)
    bW = np.asarray(bW, np.float32)
    Ubin = np.asarray(Ubin, np.float32)
    bUbin = np.asarray(bUbin, np.float32)
    Uun = np.asarray(Uun, np.float32)
    bUun = np.asarray(bUun, np.float32)

    biases = np.stack(
        [
            bW[3],                # leaf
            bW[0] + bUun[0],      # i common
            bW[1] + bUun[1],      # fL common
            bW[1] + bUbin[2] - 40.0,  # fR (binary-only; -40 kills unary)
            bW[2] + bUun[2],      # o common
            bW[3] + bUun[3],      # u common
        ]
    ).astype(np.float32)
    deltas = np.stack(
        [
            bUbin[0] - bUun[0],
            bUbin[1] - bUun[1],
            bUbin[3] - bUun[2],
            bUbin[4] - bUun[3],
            np.full(H, 40.0, np.float32),
        ]
    ).astype(BF16)

    return dict(
        emb_bf=emb.astype(BF16),
        w_bf=W.astype(BF16),
        ubt_bf=Ubin[:, :H, :].astype(BF16),
        ubb_bf=Ubin[:, H:, :].astype(BF16),
        uun_bf=Uun.astype(BF16),
        biases=biases,
        deltas=deltas,
    )


def kernel(tokens, arity, emb, W, bW, Ubin, bUbin, Uun, bUun):
    from concourse.bass_utils import run_bass_kernel_spmd

    tokens = np.asarray(tokens)
    arity = np.asarray(arity)

    shared = prep_shared_inputs(emb, W, bW, Ubin, bUbin, Uun, bUun)
    in_maps = [
        prep_core_inputs(
            tokens[k * BL : (k + 1) * BL], arity[k * BL : (k + 1) * BL], shared
        )
        for k in range(NCORES)
    ]

    nc = _build_nc()
    res = run_bass_kernel_spmd(nc, in_maps, core_ids=list(range(NCORES)))
    results = res.results

    h = np.concatenate([r["h_out"].T for r in results], axis=0)
    c = np.concatenate([r["c_out"].T for r in results], axis=0)
    return h.astype(np.float32), c.astype(np.float32)



# revision 1
# speedup vs baseline: 1.0749x; 1.0749x over previous
"""MixedArityTreeLSTM Trainium2 kernel.

Level-synchronous bottom-up Tree-LSTM over B=256 heap-indexed perfect binary
trees (511 nodes, depth 8), E=H=128. Pure data-parallel over 8 NeuronCores
(32 trees per core); all weights replicated.

Per-core layout: activations stored feature-major [H(part), nodes(free)].
Heap order makes left/right children the even/odd columns of the child level.
Binary/unary arity blending is folded into the matmuls via masked children:
    pre_g = W_g^T x + Ubt_g^T (m*h_l) + Ubb_g^T (m*h_r) + Uun_g^T ((1-m)*h_l)
            + m * (b_bin_g - b_un_g)   [K=1 outer-product matmul]
            + (bW_g + b_un_g)          [ACT bias]
Matmul operands are bf16 (2 col/cycle on PE); PSUM/h/c/gates stay fp32.
The embedding gather uses dma_gather(transpose=True) on a bf16 embedding
table, which lands x^T (feature-major) in SBUF directly.
"""

import os

import numpy as np
import ml_dtypes

# debug knobs (bisection); full kernel when unset
DBG_MIN_LVL = int(os.environ.get("TL_MIN_LVL", "0"))  # stop after this level
DBG_NO_DELTA = os.environ.get("TL_NO_DELTA", "") == "1"
N_QUEUES = int(os.environ.get("TL_NQ", "2"))

B, D = 256, 8
V, E, H = 32000, 128, 128
N_NODES = 2 ** (D + 1) - 1  # 511
NCORES = 8
BL = B // NCORES  # 32 trees per core

# levels in processing order: leaves (l=8) then 7..0
# (lvl, n real cols, Pw padded cols)
LEVELS = [(l, BL * (2**l), max(128, BL * (2**l))) for l in range(D, -1, -1)]
LVL_N = {l: BL * (2**l) for l in range(D + 1)}
LVL_PW = {l: max(128, BL * (2**l)) for l in range(D + 1)}

# chunks per level (chunk = up to 512 cols)
CPL = {l: max(1, LVL_N[l] // 512) for l in range(D + 1)}

# post-order dependency wave over the chunk tree: children before parent
ORDER = []


def _post(l, j):
    if l < D:
        if CPL[l + 1] == 2 * CPL[l]:
            _post(l + 1, 2 * j)
            _post(l + 1, 2 * j + 1)
        else:
            assert CPL[l + 1] == CPL[l] == 1
            _post(l + 1, 0)
    ORDER.append((l, j))


_post(0, 0)

# gather calls in wave order: (lvl, col0 within level's padded xT, width)
GATHER_CALLS = [
    (lvl, j * 512, min(512, LVL_PW[lvl] - j * 512)) for lvl, j in ORDER
]

# internal-level compute chunks in wave order: (cid, lvl, c0, N, mask offset)
CHUNKS = []
_moff = 0
for lvl, j in ORDER:
    if lvl == D:
        continue
    N = min(512, LVL_N[lvl] - j * 512)
    CHUNKS.append((len(CHUNKS), lvl, j * 512, N, _moff))
    _moff += N
N_MASK_ROWS = len(CHUNKS)  # 19
MASKB_LEN = _moff  # 8160

IDX_COLS = sum(w // 16 for _, _, w in GATHER_CALLS)  # 1032

BF16 = ml_dtypes.bfloat16

_CACHE = {}


def _build_nc():
    """Build the (SPMD, per-core) Bass/Tile kernel. Cached per process."""
    if "nc" in _CACHE:
        return _CACHE["nc"]

    from contextlib import ExitStack

    import concourse.mybir as mybir
    import concourse.tile as tile
    from concourse import bacc

    dt = mybir.dt
    AF = mybir.ActivationFunctionType

    nc = bacc.Bacc(num_swdge_queues=N_QUEUES)

    emb_d = nc.dram_tensor("emb_bf", [V, E], dt.bfloat16, kind="ExternalInput")
    idx_d = nc.dram_tensor("gidx", [128, IDX_COLS], dt.int16, kind="ExternalInput")
    mbc_d = nc.dram_tensor(
        "mbcast", [128, MASKB_LEN], dt.bfloat16, kind="ExternalInput"
    )
    maskb_d = nc.dram_tensor(
        "maskb", [1, MASKB_LEN], dt.bfloat16, kind="ExternalInput"
    )
    w_d = nc.dram_tensor("w_bf", [4, E, H], dt.bfloat16, kind="ExternalInput")
    ubt_d = nc.dram_tensor("ubt_bf", [5, H, H], dt.bfloat16, kind="ExternalInput")
    ubb_d = nc.dram_tensor("ubb_bf", [5, H, H], dt.bfloat16, kind="ExternalInput")
    uun_d = nc.dram_tensor("uun_bf", [4, H, H], dt.bfloat16, kind="ExternalInput")
    # bias rows: 0=b_leaf 1=bc_i 2=bc_fL 3=b_fR 4=bc_o 5=bc_u
    bias_d = nc.dram_tensor("biases", [6, H], dt.float32, kind="ExternalInput")
    # delta rows: 0=d_i 1=d_fL 2=d_o 3=d_u 4=+40 (f_r unary kill)
    delt_d = nc.dram_tensor("deltas", [5, H], dt.bfloat16, kind="ExternalInput")

    h_out_d = nc.dram_tensor("h_out", [H, BL], dt.float32, kind="ExternalOutput")
    c_out_d = nc.dram_tensor("c_out", [H, BL], dt.float32, kind="ExternalOutput")

    with tile.TileContext(nc) as tc, ExitStack() as ctx:
        consts = ctx.enter_context(tc.tile_pool(name="consts", bufs=1))

        w_sb = consts.tile([E, 4, H], dt.bfloat16)
        nc.sync.dma_start(out=w_sb, in_=w_d[:, :, :].rearrange("g e h -> e g h"))
        ubt_sb = consts.tile([H, 5, H], dt.bfloat16)
        nc.sync.dma_start(out=ubt_sb, in_=ubt_d[:, :, :].rearrange("g k h -> k g h"))
        ubb_sb = consts.tile([H, 5, H], dt.bfloat16)
        nc.sync.dma_start(out=ubb_sb, in_=ubb_d[:, :, :].rearrange("g k h -> k g h"))
        uun_sb = consts.tile([H, 4, H], dt.bfloat16)
        nc.sync.dma_start(out=uun_sb, in_=uun_d[:, :, :].rearrange("g k h -> k g h"))
        bias_sb = consts.tile([H, 6], dt.float32)
        nc.sync.dma_start(out=bias_sb, in_=bias_d[:, :].rearrange("n h -> h n"))
        delt_sb = consts.tile([1, 5, H], dt.bfloat16)
        nc.sync.dma_start(out=delt_sb, in_=delt_d[:, :].rearrange("(o g) h -> o g h", o=1))
        idx_sb = consts.tile([128, IDX_COLS], dt.int16)
        nc.sync.dma_start(out=idx_sb, in_=idx_d[:, :])
        mbc_sb = consts.tile([128, MASKB_LEN], dt.bfloat16)
        nc.sync.dma_start(out=mbc_sb, in_=mbc_d[:, :])
        maskb_sb = consts.tile([1, MASKB_LEN], dt.bfloat16)
        nc.sync.dma_start(out=maskb_sb, in_=maskb_d[:, :])

        # --- per-level xT tiles + all gathers issued up front ---
        lev = ctx.enter_context(tc.tile_pool(name="lev", bufs=1))
        xt = {}
        for lvl, n, pw in LEVELS:
            xt[lvl] = lev.tile(
                [128, pw], dt.bfloat16, name=f"xTl{lvl}", tag=f"xTl{lvl}"
            )


        # --- working pools ---
        psum = ctx.enter_context(tc.tile_pool(name="psum", bufs=8, space="PSUM"))
        work = ctx.enter_context(tc.tile_pool(name="work", bufs=2))

        h_t = {}
        c_t = {}
        n8 = LVL_N[D]
        h_t[D] = lev.tile([H, n8], dt.bfloat16, name="h_leaf", tag="h_leaf")

        # wave loop: for each chunk in post-order, gather its x then compute
        icols = {}
        _ic = 0
        for gi_, (lvl, c0, width) in enumerate(GATHER_CALLS):
            icols[(lvl, c0)] = (_ic, width, gi_)
            _ic += width // 16

        cid_of = {(lvl, c0): (cid, N, moff) for cid, lvl, c0, N, moff in CHUNKS}

        for lvl, j in ORDER:
            g0 = j * 512
            _icol, width, gi_ = icols[(lvl, g0)]
            out_view = xt[lvl][:, g0 : g0 + width].rearrange(
                "p (o n) -> p o n", o=1
            )
            nc.gpsimd.dma_gather(
                out_view,
                emb_d[:, :],
                idx_sb[:, _icol : _icol + width // 16],
                width,
                width,
                E,
                transpose=True,
                queue_num=gi_ % N_QUEUES,
            )

            if lvl == D:
                # leaf chunk: h = tanh(W3^T x + b3)
                ps = psum.tile([H, width], dt.float32, tag="pg", name="ps_leaf")
                nc.tensor.matmul(
                    ps, w_sb[:, 3, :], xt[D][:, g0 : g0 + width],
                    start=True, stop=True,
                )
                nc.scalar.activation(
                    h_t[D][:, g0 : g0 + width], ps, AF.Tanh, bias=bias_sb[:, 0:1]
                )
                continue

            cid, N, moff = cid_of[(lvl, g0)]
            c0 = g0
            if lvl < DBG_MIN_LVL:
                continue
            first_chunk = c0 == 0
            if first_chunk:
                n = LVL_N[lvl]
                hdt = dt.float32 if lvl == 0 else dt.bfloat16
                h_t[lvl] = lev.tile([H, n], hdt, name=f"h_l{lvl}", tag=f"h_l{lvl}")
                c_t[lvl] = lev.tile(
                    [H, n], dt.float32, name=f"c_l{lvl}", tag=f"c_l{lvl}"
                )

            hch = h_t[lvl + 1]
            pairs = hch[:, 2 * c0 : 2 * c0 + 2 * N].rearrange(
                "p (n two) -> p n two", two=2
            )
            h_e, h_o = pairs[:, :, 0], pairs[:, :, 1]

            mb = mbc_sb[:, moff : moff + N]

            heb = work.tile([128, N], dt.bfloat16, tag="heb", name="heb")
            nc.vector.tensor_mul(heb, h_e, mb)
            hob = work.tile([128, N], dt.bfloat16, tag="hob", name="hob")
            nc.vector.tensor_mul(hob, h_o, mb)
            heu = work.tile([128, N], dt.bfloat16, tag="heu", name="heu")
            nc.vector.tensor_sub(heu, h_e, heb)

            xs = xt[lvl][:, c0 : c0 + N]
            mrow = maskb_sb[:, moff : moff + N]
            top = lvl == D - 1  # children are leaves: c=0, skip f gates

            # gate -> (W idx, Ubin idx, Uun idx or None, delta idx or None)
            if top:
                gates = [("i", 0, 0, 0, 0), ("o", 2, 3, 2, 2), ("u", 3, 4, 3, 3)]
            else:
                gates = [
                    ("i", 0, 0, 0, 0),
                    ("fl", 1, 1, 1, 1),
                    ("fr", 1, 2, None, 4),
                    ("o", 2, 3, 2, 2),
                    ("u", 3, 4, 3, 3),
                ]

            pts = {}
            for gname, wi, ubi, uui, di in gates:
                ps = psum.tile([H, N], dt.float32, tag="pg", name=f"ps_{gname}")
                nc.tensor.matmul(ps, w_sb[:, wi, :], xs, start=True, stop=False)
                nc.tensor.matmul(ps, ubt_sb[:, ubi, :], heb, start=False, stop=False)
                nc.tensor.matmul(
                    ps, ubb_sb[:, ubi, :], hob,
                    start=False, stop=(DBG_NO_DELTA and uui is None),
                )
                if uui is not None:
                    nc.tensor.matmul(
                        ps, uun_sb[:, uui, :], heu, start=False, stop=DBG_NO_DELTA
                    )
                if not DBG_NO_DELTA:
                    nc.tensor.matmul(
                        ps, delt_sb[:, di, :], mrow, start=False, stop=True
                    )
                pts[gname] = ps

            gi = work.tile([128, N], dt.float32, tag="gi", name="gi")
            nc.scalar.activation(gi, pts["i"], AF.Sigmoid, bias=bias_sb[:, 1:2])
            go = work.tile([128, N], dt.float32, tag="go", name="go")
            nc.scalar.activation(go, pts["o"], AF.Sigmoid, bias=bias_sb[:, 4:5])
            gu = work.tile([128, N], dt.float32, tag="gu", name="gu")
            nc.scalar.activation(gu, pts["u"], AF.Tanh, bias=bias_sb[:, 5:6])

            cs = c_t[lvl][:, c0 : c0 + N]
            if top:
                nc.vector.tensor_mul(cs, gi, gu)
            else:
                gfl = work.tile([128, N], dt.float32, tag="gfl", name="gfl")
                nc.scalar.activation(
                    gfl, pts["fl"], AF.Sigmoid, bias=bias_sb[:, 2:3]
                )
                gfr = work.tile([128, N], dt.float32, tag="gfr", name="gfr")
                nc.scalar.activation(
                    gfr, pts["fr"], AF.Sigmoid, bias=bias_sb[:, 3:4]
                )
                cch = c_t[lvl + 1]
                cpairs = cch[:, 2 * c0 : 2 * c0 + 2 * N].rearrange(
                    "p (n two) -> p n two", two=2
                )
                c_e, c_o = cpairs[:, :, 0], cpairs[:, :, 1]

                t1 = work.tile([128, N], dt.float32, tag="t1", name="t1")
                nc.vector.tensor_mul(t1, gi, gu)
                t2 = work.tile([128, N], dt.float32, tag="t2", name="t2")
                nc.vector.tensor_mul(t2, gfl, c_e)
                nc.vector.tensor_add(cs, t1, t2)
                t3 = work.tile([128, N], dt.float32, tag="t3", name="t3")
                nc.vector.tensor_mul(t3, gfr, c_o)
                nc.vector.tensor_add(cs, cs, t3)

            tch = work.tile([128, N], dt.float32, tag="tch", name="tch")
            nc.scalar.activation(tch, cs, AF.Tanh)
            nc.vector.tensor_mul(h_t[lvl][:, c0 : c0 + N], go, tch)

        ol = DBG_MIN_LVL
        h_fin = h_t[ol][:, :BL]
        c_fin = c_t[ol][:, :BL] if ol in c_t else h_t[ol][:, :BL]
        eng = nc.sync if ol == 0 else nc.gpsimd
        eng.dma_start(out=h_out_d[:, :], in_=h_fin)
        eng.dma_start(out=c_out_d[:, :], in_=c_fin)

    nc.finalize()
    _CACHE["nc"] = nc
    return nc


def _wrap_idx(seg):
    """dma_gather index layout: unwrapped[i] = idxs[i % 16, i // 16],
    replicated across the 128 partitions."""
    w = seg.reshape(-1, 16).T.astype(np.int16)  # [16, len/16]
    return np.tile(w, (8, 1))


def prep_core_inputs(tokens_c, arity_c, shared):
    """Per-core input map. tokens_c [BL,511], arity_c [BL,255]."""
    idx_cols = []
    for lvl, c0, width in GATHER_CALLS:
        off, cnt = 2**lvl - 1, 2**lvl
        flat = np.asarray(tokens_c[:, off : off + cnt]).reshape(-1)
        pw = LVL_PW[lvl]
        if pw != flat.size:
            flat = np.concatenate([flat, np.zeros(pw - flat.size, np.int64)])
        idx_cols.append(_wrap_idx(flat[c0 : c0 + width]))
    gidx = np.concatenate(idx_cols, axis=1)
    assert gidx.shape == (128, IDX_COLS)

    maskb = np.zeros((1, MASKB_LEN), BF16)
    for cid, lvl, c0, N, moff in CHUNKS:
        off = 2**lvl - 1
        m = (
            (np.asarray(arity_c[:, off : off + 2**lvl]).reshape(-1) == 1)
            .astype(np.float32)
        )
        maskb[0, moff : moff + N] = m[c0 : c0 + N].astype(BF16)

    return dict(
        shared,
        gidx=gidx,
        maskb=maskb,
        mbcast=np.broadcast_to(maskb, (128, MASKB_LEN)).copy(),
    )


def prep_shared_inputs(emb, W, bW, Ubin, bUbin, Uun, bUun):
    emb = np.asarray(emb, np.float32)
    W = np.asarray(W, np.float32)
    bW = np.asarray(bW, np.float32)
    Ubin = np.asarray(Ubin, np.float32)
    bUbin = np.asarray(bUbin, np.float32)
    Uun = np.asarray(Uun, np.float32)
    bUun = np.asarray(bUun, np.float32)

    biases = np.stack(
        [
            bW[3],                # leaf
            bW[0] + bUun[0],      # i common
            bW[1] + bUun[1],      # fL common
            bW[1] + bUbin[2] - 40.0,  # fR (binary-only; -40 kills unary)
            bW[2] + bUun[2],      # o common
            bW[3] + bUun[3],      # u common
        ]
    ).astype(np.float32)
    deltas = np.stack(
        [
            bUbin[0] - bUun[0],
            bUbin[1] - bUun[1],
            bUbin[3] - bUun[2],
            bUbin[4] - bUun[3],
            np.full(H, 40.0, np.float32),
        ]
    ).astype(BF16)

    return dict(
        emb_bf=emb.astype(BF16),
        w_bf=W.astype(BF16),
        ubt_bf=Ubin[:, :H, :].astype(BF16),
        ubb_bf=Ubin[:, H:, :].astype(BF16),
        uun_bf=Uun.astype(BF16),
        biases=biases,
        deltas=deltas,
    )


def kernel(tokens, arity, emb, W, bW, Ubin, bUbin, Uun, bUun):
    from concourse.bass_utils import run_bass_kernel_spmd

    tokens = np.asarray(tokens)
    arity = np.asarray(arity)

    shared = prep_shared_inputs(emb, W, bW, Ubin, bUbin, Uun, bUun)
    in_maps = [
        prep_core_inputs(
            tokens[k * BL : (k + 1) * BL], arity[k * BL : (k + 1) * BL], shared
        )
        for k in range(NCORES)
    ]

    nc = _build_nc()
    res = run_bass_kernel_spmd(nc, in_maps, core_ids=list(range(NCORES)))
    results = res.results

    h = np.concatenate([r["h_out"].T for r in results], axis=0)
    c = np.concatenate([r["c_out"].T for r in results], axis=0)
    return h.astype(np.float32), c.astype(np.float32)

